# revision 4
# baseline (speedup 1.0000x reference)
"""Adversarial loss kernel for Trainium2 (8 NeuronCores, data-parallel).

For pred [4096, 32000] f32 and target [4096] int:
    out[b] = -(sum_c log(sigmoid(pred[b,c])) - log(sigmoid(pred[b,target[b]]))) / C

Sharding: pure data parallel over the batch dim — 512 rows per core.

Per-core pipeline (memory-bound problem; ~65.5 MB of pred per core):
  1. DMA [128, CT] tiles of pred into SBUF.  The sync HWDGE queue carries
     ONLY these bulk tiles: every [P,1]-shaped transfer (index loads,
     output writes) would otherwise spray 4-byte packets into the same
     queue and delay the first bulk packet by ~2.5us.
  2. ScalarE ACT computes sigmoid(x) per tile — one activation function
     for the bulk pass, so the ACT table stays resident.
  3. VectorE reduces groups of 16 sigmoids with a product (ln prod sigma =
     sum ln sigma; groups of 16 keep the product far above the ~2^-64
     LN-table clamp).
  4. The target entry of each row is fetched by indirect-gather DMA at the
     start; 1/sigmoid(x_t) is appended as one extra product column — its
     ln contributes exactly -ln sigmoid(x_t).  The whole correction chain
     is emitted right after row block 0 so it never lands in the tail.
  5. LN+accumulate per row block over the product columns yields
     sum_c ln sigmoid - ln sigmoid_t; scale by -1/C.  LNs are batched in
     two table visits (rb 0-2 mid-stream, rb 3 in the drain) to halve the
     Sigmoid<->Ln table-swap traffic.  The last column tile of the last
     row block is split so the post-DMA pipeline drain is short.
"""

import sys

sys.path.insert(0, "/opt/trn_rl_repo")

import numpy as np

from concourse import bass, bacc, mybir
import concourse.tile as tile
from concourse.bass_utils import run_bass_kernel_spmd

B, C = 4096, 32000
NCORES = 8
R = B // NCORES  # rows per core
P = 128  # SBUF partitions
NRB = R // P  # row blocks per core

# Tunables (overridable via build_nc kwargs for experiments; the defaults
# are the tuned configuration used for grading).
CT = 2000  # column-tile width
# Product-group size: ln(prod of GRP sigmoids) must stay far above ~2^-64,
# where the ScalarE LN table clamps (HW-measured).  GRP=16 keeps group
# products >= ~1e-12 for randn inputs (>10 sigma of margin); GRP=40 was
# observed to dip below the clamp and corrupt rows.
GRP = 16
PIN_BUFS = 8
PSG_BUFS = 6
TAIL_SPLIT = 240  # width of the final (drain-shortening) tile, multiple of GRP

F32 = mybir.dt.float32
I32 = mybir.dt.int32
SIG = mybir.ActivationFunctionType.Sigmoid
LN = mybir.ActivationFunctionType.Ln


def build_nc(ct=None, grp=None, pin_bufs=None, psg_bufs=None, tail_split=None):
    ct = CT if ct is None else ct
    grp = GRP if grp is None else grp
    pin_bufs = PIN_BUFS if pin_bufs is None else pin_bufs
    psg_bufs = PSG_BUFS if psg_bufs is None else psg_bufs
    tail_split = TAIL_SPLIT if tail_split is None else tail_split

    nct = C // ct  # column tiles per row block
    ng = ct // grp  # product columns per full tile
    ngr = C // grp  # product columns per row block
    assert tail_split % grp == 0 and 0 < tail_split < ct

    nc = bacc.Bacc(None, target_bir_lowering=False)
    pred = nc.declare_dram_parameter("pred", [R, C], F32, isOutput=False)
    gidx = nc.declare_dram_parameter("gidx", [R], I32, isOutput=False)
    out = nc.declare_dram_parameter("out", [R], F32, isOutput=True)

    # Flat [R*C, 1] view of pred for the target-element gather.
    pred_flat = pred[:, :].rearrange("a b -> (a b)")[:, None]

    with tile.TileContext(nc) as tc:
        with (
            tc.tile_pool(name="pin", bufs=pin_bufs) as pin,
            tc.tile_pool(name="psg", bufs=psg_bufs) as psg,
            tc.tile_pool(name="pg", bufs=1) as pg,
            tc.tile_pool(name="pln", bufs=2) as pln,
            tc.tile_pool(name="psm", bufs=2) as psm,
        ):
            # Gather pred[r, target[r]] for all rows: one [P, 1] indirect
            # DMA per row block into a shared [P, NRB] tile.  Index loads
            # ride the scalar HWDGE queue, NOT sync (see module docstring).
            # The memset bounds the damage if a gather ever lands late.
            tv = psm.tile([P, NRB], F32, tag="tv")
            nc.gpsimd.memset(tv[:], 0.0)
            for rb in range(NRB):
                idx_t = psm.tile([P, 1], I32, tag=f"idx{rb}")
                nc.scalar.dma_start(
                    out=idx_t[:], in_=gidx[rb * P : (rb + 1) * P, None]
                )
                nc.gpsimd.indirect_dma_start(
                    out=tv[:, rb : rb + 1],
                    out_offset=None,
                    in_=pred_flat,
                    in_offset=bass.IndirectOffsetOnAxis(ap=idx_t[:, :1], axis=0),
                )

            # One product tile per row block: ngr group products plus one
            # correction column holding 1/sigmoid(x_t).
            gt = []
            for rb in range(NRB):
                g_rb = pg.tile([P, ngr + 1], F32, name=f"g{rb}", tag=f"g{rb}")
                gt.append(g_rb)

            # Column-tile widths: full tiles, except the last tile of the
            # last row block is split so the post-DMA drain is short.
            def col_tiles(rb):
                tiles = [(i * ct, ct) for i in range(nct)]
                if rb == NRB - 1:
                    last_off, _ = tiles[-1]
                    tiles[-1] = (last_off, ct - tail_split)
                    tiles.append((last_off + ct - tail_split, tail_split))
                return tiles

            def bulk_tile(rb, c0, w):
                rows = slice(rb * P, (rb + 1) * P)
                t = pin.tile([P, w], F32, name="tin", tag="in")
                nc.sync.dma_start(out=t[:], in_=pred[rows, c0 : c0 + w])
                s = psg.tile([P, w], F32, name="tsig", tag="sig")
                nc.scalar.activation(out=s[:], in_=t[:], func=SIG)
                g0 = c0 // grp
                nc.vector.tensor_reduce(
                    out=gt[rb][:, g0 : g0 + w // grp],
                    in_=s[:].rearrange("p (g k) -> p g k", k=grp),
                    op=mybir.AluOpType.mult,
                    axis=mybir.AxisListType.X,
                )

            def ln_block(rb):
                rows = slice(rb * P, (rb + 1) * P)
                lnout = pln.tile([P, ngr + 1], F32, name="lnout", tag="lnout")
                acc = psm.tile([P, 1], F32, name="acc", tag="acc")
                nc.scalar.activation(
                    out=lnout[:], in_=gt[rb][:], func=LN, accum_out=acc[:]
                )
                o = psm.tile([P, 1], F32, name="o", tag="o")
                nc.scalar.mul(o[:], acc[:], -1.0 / C)
                nc.scalar.dma_start(out=out[rows, None], in_=o[:])

            for rb in range(NRB):
                for c0, w in col_tiles(rb):
                    bulk_tile(rb, c0, w)
                if rb == 0:
                    # Correction terms, emitted right after row block 0 so
                    # they are long done before the drain: 1/sigmoid(x_t)
                    # goes into each row block's extra product column (its
                    # ln contributes exactly -ln sigmoid(x_t)).
                    sgt = psm.tile([P, NRB], F32, tag="sgt")
                    nc.scalar.activation(out=sgt[:], in_=tv[:], func=SIG)
                    rec = psm.tile([P, NRB], F32, tag="rec")
                    nc.vector.reciprocal(out=rec[:], in_=sgt[:])
                    with nc.allow_low_precision("correction col; ~1e-7 rel"):
                        for rb2 in range(NRB):
                            nc.vector.tensor_copy(
                                out=gt[rb2][:, ngr : ngr + 1],
                                in_=rec[:, rb2 : rb2 + 1],
                            )
                if rb == NRB - 2:
                    # One Ln-table visit for row blocks 0..NRB-2; only the
                    # last row block's LN lands in the drain.
                    for rb2 in range(NRB - 1):
                        ln_block(rb2)
            ln_block(NRB - 1)
    nc.finalize()
    return nc


_NC = None


def _get_nc():
    global _NC
    if _NC is None:
        _NC = build_nc()
    return _NC


def _make_in_maps(pred, target):
    pred = np.ascontiguousarray(np.asarray(pred, dtype=np.float32))
    tgt = np.asarray(target).astype(np.int64)
    in_maps = []
    for c in range(NCORES):
        rs = c * R
        loc_t = tgt[rs : rs + R]
        g = (np.arange(R, dtype=np.int64) * C + loc_t).astype(np.int32)
        in_maps.append({"pred": pred[rs : rs + R], "gidx": g})
    return in_maps


def kernel(pred, target, _trace=False):
    nc = _get_nc()
    in_maps = _make_in_maps(pred, target)
    res = run_bass_kernel_spmd(
        nc, in_maps, core_ids=list(range(NCORES)), trace=_trace
    )
    out = np.concatenate([res.results[i]["out"] for i in range(NCORES)])
    if _trace:
        kernel.last_results = res
    return out.astype(np.float32)
